# revision 32
# baseline (speedup 1.0000x reference)
"""Batched-A Trainium2 kernel for nn_Dilate (7x7 ones conv -> y>0 int32 mask).

Structure (per core: 2 images x 9 row-tiles, batches [1,3,3,2]/[3,3,2,1]):
  - Inputs load per BATCH: one HWDGE issue per uniform-stride run of tiles
    (a custom overlapping 3D access pattern re-reads the 6 halo rows), so
    the sync ring issues ~10 DMAs instead of 18 -- the issue serialization
    was throttling pipeline fill.
  - PE: banded-ones f32r matmul -> V in PSUM (2x512 cols per tile).
  - ACT: copies each tile's V into its section of a concatenated buffer
    vbig[128, nchunk*1034]; section layout [7 zeros][1024 V][3 zeros].
  - DVE: ONE boxsum sliding scan per batch over the whole vbig:
        state[t] = (vbig[t+7] + state) - vbig[t]
    The zero seams make the running window sum correct across sections
    (leading-7 zeros telescope away), so one instruction computes every
    tile's horizontal 7-tap including image edges.  DVE is the wall:
    ~41us of scan at ~2.2ns/col, unavoidable (scans are DVE-only).
  - ACT: ONE sigmoid(1e8*d) per batch over a strided 3D view -> int8 0/1.
  - Pool: per-tile SWDGE stores (HWDGE corrupts the batched 3D-AP stores,
    and per-tile granularity keeps the end-of-program queue drains short).
  - Emission interleaves the ACT stream as C0 C1 C2 S0 C3 S1 ... so
    sigmoids never head-block upcoming copies (streams execute in order).
  - Small first batch -> first scan starts early; small last batch ->
    short drain tail.
"""

import numpy as np

import concourse.bacc as bacc
import concourse.mybir as mybir
from concourse.ap import AP
from concourse.tile import TileContext
from concourse.bass_utils import run_bass_kernel_spmd

B, H, W = 16, 1024, 1024
NCORES = 8
PER_CORE = B // NCORES  # 2 images per core
R = 7
PAD = R // 2  # 3
P = 128             # SBUF partitions per tile (input rows incl. halo)
MOUT = P - (R - 1)  # 122 output rows per tile
NTILES = -(-H // MOUT)  # 9 row tiles per image
L = R + W + PAD     # 1034: per-tile section length in the scan buffer

SIG_SCALE = 1.0e8
# per-image batch sizes: small first batch -> the first scan starts early
# (pipeline fill), small last batch -> short drain tail.
BATCHES_PER_IMG = [[1, 2, 3, 3], [3, 3, 2, 1]]
N_XG = 3            # rotating grouped-x buffers
N_VB = 4            # rotating vbig buffers
N_DB = 3            # rotating dbig buffers
N_MB = 3            # rotating mask buffers


def _band_matrices() -> np.ndarray:
    bands = np.zeros((3, P, MOUT), dtype=np.float32)
    for m in range(MOUT):
        bands[0, max(0, m - PAD) : m + PAD + 1, m] = 1.0
        bands[1, m : m + R, m] = 1.0
    for m in range(48):
        bands[2, 80 + m - PAD : min(80 + m + PAD + 1, P), m] = 1.0
    return bands


def _build_program():
    nc = bacc.Bacc("TRN2")
    x_d = nc.dram_tensor("x", [PER_CORE, H, W], mybir.dt.float32, kind="ExternalInput")
    band_d = nc.dram_tensor("band", [3, P, MOUT], mybir.dt.float32r, kind="ExternalInput")
    y_d = nc.dram_tensor("y", [PER_CORE, H, W], mybir.dt.int8, kind="ExternalOutput")

    add = mybir.AluOpType.add
    sub = mybir.AluOpType.subtract
    sig = mybir.ActivationFunctionType.Sigmoid
    f32 = mybir.dt.float32
    f32r = mybir.dt.float32r

    # tiles[i] = (band_idx, img, row_lo, o0, nvalid)
    tiles = []
    for img in range(PER_CORE):
        for t in range(NTILES):
            o0 = t * MOUT
            lo = 0 if t == 0 else (H - P if t == NTILES - 1 else o0 - PAD)
            tiles.append((0 if t == 0 else (2 if t == NTILES - 1 else 1),
                          img, lo, o0, min(MOUT, H - o0)))

    # batches[k] = (img, t0, nchunk)
    batches = []
    for img in range(PER_CORE):
        t0 = 0
        for n in BATCHES_PER_IMG[img]:
            batches.append((img, t0, n))
            t0 += n
        assert t0 == NTILES
    NB = len(batches)
    MAXCH = max(n for _, _, n in batches)

    with TileContext(nc) as tc:
        with (
            tc.tile_pool(name="const", bufs=1) as cpool,
            tc.tile_pool(name="xg", bufs=N_XG) as xgpool,
            tc.tile_pool(name="psum", bufs=4, space="PSUM") as psum_pool,
        ):
            band_ts = []
            for i in range(3):
                bt = cpool.tile([P, MOUT], f32r, tag=f"band{i}")
                nc.scalar.dma_start(out=bt[:], in_=band_d[i])
                band_ts.append(bt)

            vbig, dbig, mbuf = [], [], []
            for i in range(N_VB):
                vb = cpool.tile([P, MAXCH * L], f32, tag=f"vb{i}")
                nc.gpsimd.memset(vb[:MOUT, 0:R], 0.0)
                for s in range(MAXCH - 1):
                    nc.gpsimd.memset(vb[:MOUT, s * L + R + W : (s + 1) * L + R], 0.0)
                nc.gpsimd.memset(
                    vb[:MOUT, (MAXCH - 1) * L + R + W : MAXCH * L], 0.0
                )
                vbig.append(vb)
            for i in range(N_DB):
                db = cpool.tile([P, MAXCH * L], f32, tag=f"db{i}")
                dbig.append(db)
            for i in range(N_MB):
                mb = cpool.tile([P, MAXCH * W], mybir.dt.int8, tag=f"mb{i}")
                mbuf.append(mb)

            # grouped input loads: one sync-HWDGE issue per uniform-stride
            # run of tiles inside a batch (overlapping 3D src AP re-reads
            # the halo rows; DRAM reads may overlap freely).
            xbufs = []
            for k, (img, t0, nchunk) in enumerate(batches):
                xb = xgpool.tile([P, MAXCH * W], f32r)
                los = [tiles[img * NTILES + t0 + s][2] for s in range(nchunk)]
                s = 0
                while s < nchunk:
                    r = 1
                    while s + r < nchunk and los[s + r] - los[s + r - 1] == MOUT:
                        r += 1
                    if r == 1:
                        nc.sync.dma_start(
                            out=xb[:, s * W : (s + 1) * W],
                            in_=x_d[img, los[s] : los[s] + P, :].bitcast(f32r),
                        )
                    else:
                        src = AP(
                            tensor=x_d,
                            offset=(img * H + los[s]) * W,
                            ap=[[W, P], [MOUT * W, r], [1, W]],
                        ).bitcast(f32r)
                        nc.sync.dma_start(
                            out=xb[:, s * W : (s + r) * W]
                            .rearrange("p (t c) -> p t c", t=r),
                            in_=src,
                        )
                    s += r
                xbufs.append(xb)

            def emit_mm_copies(k):
                img, t0, nchunk = batches[k]
                vb = vbig[k % N_VB]
                xb = xbufs[k]
                for s in range(nchunk):
                    ti = img * NTILES + t0 + s
                    bt = band_ts[tiles[ti][0]]
                    v_ps = psum_pool.tile([MOUT, W], f32)
                    for j in range(2):
                        nc.tensor.matmul(
                            v_ps[:, j * 512 : (j + 1) * 512],
                            bt[:],
                            xb[:, s * W + j * 512 : s * W + (j + 1) * 512],
                            start=True,
                            stop=True,
                        )
                    nc.scalar.copy(vb[:MOUT, s * L + R : s * L + R + W], v_ps[:])

            def emit_scan(k):
                img, t0, nchunk = batches[k]
                vb, db = vbig[k % N_VB], dbig[k % N_DB]
                n = nchunk * L - R
                nc.vector.tensor_tensor_scan(
                    db[:MOUT, 0:n],
                    vb[:MOUT, R : R + n],
                    vb[:MOUT, 0:n],
                    0.0,
                    add,
                    sub,
                )

            def emit_sigmoid_store(k):
                img, t0, nchunk = batches[k]
                db, mb = dbig[k % N_DB], mbuf[k % N_MB]
                # section s, output col j lives at db col s*L + PAD + j
                if nchunk > 1:
                    nc.scalar.activation(
                        mb[:MOUT, 0 : nchunk * W].rearrange("m (c w) -> m c w", c=nchunk),
                        db[:MOUT, 0 : nchunk * L]
                        .rearrange("m (c l) -> m c l", c=nchunk)[:, :, PAD : PAD + W],
                        sig,
                        scale=SIG_SCALE,
                    )
                else:
                    nc.scalar.activation(
                        mb[:MOUT, 0:W],
                        db[:MOUT, PAD : PAD + W],
                        sig,
                        scale=SIG_SCALE,
                    )
                # per-tile SWDGE stores (int8) -- HWDGE corrupts int8 stores
                # in this stack (2-of-4-byte granule pattern), so all output
                # goes through the software DGE
                for s in range(nchunk):
                    _, _, _, o0s, nv = tiles[img * NTILES + t0 + s]
                    nc.gpsimd.dma_start(
                        out=y_d[img, o0s : o0s + nv, :],
                        in_=mb[:nv, s * W : s * W + W],
                    )

            # software pipeline: ACT stream = C0 C1 C2 S0 C3 S1 C4 S2 ...
            emit_mm_copies(0)
            for k in range(1, NB):
                emit_mm_copies(k)
                emit_scan(k - 1)
                if k >= 2:
                    emit_sigmoid_store(k - 2)
            emit_scan(NB - 1)
            emit_sigmoid_store(NB - 2)
            emit_sigmoid_store(NB - 1)

    nc.compile()
    return nc


_PROGRAM_CACHE = {}


def _get_program():
    if "nc" not in _PROGRAM_CACHE:
        _PROGRAM_CACHE["nc"] = _build_program()
    return _PROGRAM_CACHE["nc"]


def kernel(x, weight=None, **_unused):
    x = np.ascontiguousarray(np.asarray(x), dtype=np.float32)
    assert x.shape == (B, 1, H, W), x.shape
    xs = x.reshape(B, H, W)
    band = _band_matrices()

    nc = _get_program()
    in_maps = [
        {"x": np.ascontiguousarray(xs[c * PER_CORE : (c + 1) * PER_CORE]), "band": band}
        for c in range(NCORES)
    ]
    res = run_bass_kernel_spmd(nc, in_maps, core_ids=list(range(NCORES)))
    out = np.concatenate([r["y"] for r in res.results], axis=0)
    return out.reshape(B, 1, H, W).astype(np.int32)


# revision 34
# speedup vs baseline: 1.0373x; 1.0373x over previous
"""Batched-A Trainium2 kernel for nn_Dilate (7x7 ones conv -> y>0 int32 mask).

Structure (per core: 2 images x 9 row-tiles, batches [1,3,3,2]/[3,3,2,1]):
  - Inputs load per BATCH: one HWDGE issue per uniform-stride run of tiles
    (a custom overlapping 3D access pattern re-reads the 6 halo rows), so
    the sync ring issues ~10 DMAs instead of 18 -- the issue serialization
    was throttling pipeline fill.
  - PE: banded-ones f32r matmul -> V in PSUM (2x512 cols per tile).
  - ACT: copies each tile's V into its section of a concatenated buffer
    vbig[128, nchunk*1034]; section layout [7 zeros][1024 V][3 zeros].
  - DVE: ONE boxsum sliding scan per batch over the whole vbig:
        state[t] = (vbig[t+7] + state) - vbig[t]
    The zero seams make the running window sum correct across sections
    (leading-7 zeros telescope away), so one instruction computes every
    tile's horizontal 7-tap including image edges.  DVE is the wall:
    ~41us of scan at ~2.2ns/col, unavoidable (scans are DVE-only).
  - ACT: ONE sigmoid(1e8*d) per batch over a strided 3D view -> int8 0/1.
  - Pool: per-tile SWDGE stores (HWDGE corrupts the batched 3D-AP stores,
    and per-tile granularity keeps the end-of-program queue drains short).
  - Emission interleaves the ACT stream as C0 C1 C2 S0 C3 S1 ... so
    sigmoids never head-block upcoming copies (streams execute in order).
  - Small first batch -> first scan starts early; small last batch ->
    short drain tail.
"""

import numpy as np

import concourse.bacc as bacc
import concourse.mybir as mybir
from concourse.ap import AP
from concourse.tile import TileContext
from concourse.bass_utils import run_bass_kernel_spmd

B, H, W = 16, 1024, 1024
NCORES = 8
PER_CORE = B // NCORES  # 2 images per core
R = 7
PAD = R // 2  # 3
P = 128             # SBUF partitions per tile (input rows incl. halo)
MOUT = P - (R - 1)  # 122 output rows per tile
NTILES = -(-H // MOUT)  # 9 row tiles per image
L = R + W + PAD     # 1034: per-tile section length in the scan buffer

SIG_SCALE = 1.0e8
# per-image batch sizes: small first batch -> the first scan starts early
# (pipeline fill), small last batch -> short drain tail.
BATCHES_PER_IMG = [[1, 3, 3, 2], [3, 3, 2, 1]]
N_XG = 4            # rotating grouped-x buffers
N_VB = 4            # rotating vbig buffers
N_DB = 3            # rotating dbig buffers
N_MB = 3            # rotating mask buffers


def _band_matrices() -> np.ndarray:
    bands = np.zeros((3, P, MOUT), dtype=np.float32)
    for m in range(MOUT):
        bands[0, max(0, m - PAD) : m + PAD + 1, m] = 1.0
        bands[1, m : m + R, m] = 1.0
    for m in range(48):
        bands[2, 80 + m - PAD : min(80 + m + PAD + 1, P), m] = 1.0
    return bands


def _build_program():
    nc = bacc.Bacc("TRN2")
    x_d = nc.dram_tensor("x", [PER_CORE, H, W], mybir.dt.float32, kind="ExternalInput")
    band_d = nc.dram_tensor("band", [3, P, MOUT], mybir.dt.float32r, kind="ExternalInput")
    y_d = nc.dram_tensor("y", [PER_CORE, H, W], mybir.dt.int8, kind="ExternalOutput")

    add = mybir.AluOpType.add
    sub = mybir.AluOpType.subtract
    sig = mybir.ActivationFunctionType.Sigmoid
    f32 = mybir.dt.float32
    f32r = mybir.dt.float32r

    # tiles[i] = (band_idx, img, row_lo, o0, nvalid)
    tiles = []
    for img in range(PER_CORE):
        for t in range(NTILES):
            o0 = t * MOUT
            lo = 0 if t == 0 else (H - P if t == NTILES - 1 else o0 - PAD)
            tiles.append((0 if t == 0 else (2 if t == NTILES - 1 else 1),
                          img, lo, o0, min(MOUT, H - o0)))

    # batches[k] = (img, t0, nchunk)
    batches = []
    for img in range(PER_CORE):
        t0 = 0
        for n in BATCHES_PER_IMG[img]:
            batches.append((img, t0, n))
            t0 += n
        assert t0 == NTILES
    NB = len(batches)
    MAXCH = max(n for _, _, n in batches)

    with TileContext(nc) as tc:
        with (
            tc.tile_pool(name="const", bufs=1) as cpool,
            tc.tile_pool(name="xg", bufs=N_XG) as xgpool,
            tc.tile_pool(name="psum", bufs=4, space="PSUM") as psum_pool,
        ):
            band_ts = []
            for i in range(3):
                bt = cpool.tile([P, MOUT], f32r, tag=f"band{i}")
                nc.scalar.dma_start(out=bt[:], in_=band_d[i])
                band_ts.append(bt)

            vbig, dbig, mbuf = [], [], []
            for i in range(N_VB):
                vb = cpool.tile([P, MAXCH * L], f32, tag=f"vb{i}")
                nc.gpsimd.memset(vb[:MOUT, 0:R], 0.0)
                for s in range(MAXCH - 1):
                    nc.gpsimd.memset(vb[:MOUT, s * L + R + W : (s + 1) * L + R], 0.0)
                nc.gpsimd.memset(
                    vb[:MOUT, (MAXCH - 1) * L + R + W : MAXCH * L], 0.0
                )
                vbig.append(vb)
            for i in range(N_DB):
                db = cpool.tile([P, MAXCH * L], f32, tag=f"db{i}")
                dbig.append(db)
            for i in range(N_MB):
                mb = cpool.tile([P, MAXCH * W], mybir.dt.int8, tag=f"mb{i}")
                mbuf.append(mb)

            # grouped input loads: one sync-HWDGE issue per uniform-stride
            # run of tiles inside a batch (overlapping 3D src AP re-reads
            # the halo rows; DRAM reads may overlap freely).
            xbufs = []
            for k, (img, t0, nchunk) in enumerate(batches):
                xb = xgpool.tile([P, MAXCH * W], f32r)
                los = [tiles[img * NTILES + t0 + s][2] for s in range(nchunk)]
                s = 0
                while s < nchunk:
                    r = 1
                    while s + r < nchunk and los[s + r] - los[s + r - 1] == MOUT:
                        r += 1
                    if r == 1:
                        nc.sync.dma_start(
                            out=xb[:, s * W : (s + 1) * W],
                            in_=x_d[img, los[s] : los[s] + P, :].bitcast(f32r),
                        )
                    else:
                        src = AP(
                            tensor=x_d,
                            offset=(img * H + los[s]) * W,
                            ap=[[W, P], [MOUT * W, r], [1, W]],
                        ).bitcast(f32r)
                        nc.sync.dma_start(
                            out=xb[:, s * W : (s + r) * W]
                            .rearrange("p (t c) -> p t c", t=r),
                            in_=src,
                        )
                    s += r
                xbufs.append(xb)

            def emit_mm_copies(k):
                img, t0, nchunk = batches[k]
                vb = vbig[k % N_VB]
                xb = xbufs[k]
                for s in range(nchunk):
                    ti = img * NTILES + t0 + s
                    bt = band_ts[tiles[ti][0]]
                    v_ps = psum_pool.tile([MOUT, W], f32)
                    for j in range(2):
                        nc.tensor.matmul(
                            v_ps[:, j * 512 : (j + 1) * 512],
                            bt[:],
                            xb[:, s * W + j * 512 : s * W + (j + 1) * 512],
                            start=True,
                            stop=True,
                        )
                    nc.scalar.copy(vb[:MOUT, s * L + R : s * L + R + W], v_ps[:])

            def emit_scan(k):
                img, t0, nchunk = batches[k]
                vb, db = vbig[k % N_VB], dbig[k % N_DB]
                n = nchunk * L - R
                nc.vector.tensor_tensor_scan(
                    db[:MOUT, 0:n],
                    vb[:MOUT, R : R + n],
                    vb[:MOUT, 0:n],
                    0.0,
                    add,
                    sub,
                )

            def emit_sigmoid_store(k):
                img, t0, nchunk = batches[k]
                db, mb = dbig[k % N_DB], mbuf[k % N_MB]
                # section s, output col j lives at db col s*L + PAD + j
                if nchunk > 1:
                    nc.scalar.activation(
                        mb[:MOUT, 0 : nchunk * W].rearrange("m (c w) -> m c w", c=nchunk),
                        db[:MOUT, 0 : nchunk * L]
                        .rearrange("m (c l) -> m c l", c=nchunk)[:, :, PAD : PAD + W],
                        sig,
                        scale=SIG_SCALE,
                    )
                else:
                    nc.scalar.activation(
                        mb[:MOUT, 0:W],
                        db[:MOUT, PAD : PAD + W],
                        sig,
                        scale=SIG_SCALE,
                    )
                # per-tile SWDGE stores (int8) -- HWDGE corrupts int8 stores
                # in this stack (2-of-4-byte granule pattern), so all output
                # goes through the software DGE
                for s in range(nchunk):
                    _, _, _, o0s, nv = tiles[img * NTILES + t0 + s]
                    nc.gpsimd.dma_start(
                        out=y_d[img, o0s : o0s + nv, :],
                        in_=mb[:nv, s * W : s * W + W],
                    )

            # software pipeline: ACT stream = C0 C1 C2 S0 C3 S1 C4 S2 ...
            emit_mm_copies(0)
            for k in range(1, NB):
                emit_mm_copies(k)
                emit_scan(k - 1)
                if k >= 2:
                    emit_sigmoid_store(k - 2)
            emit_scan(NB - 1)
            emit_sigmoid_store(NB - 2)
            emit_sigmoid_store(NB - 1)

    nc.compile()
    return nc


_PROGRAM_CACHE = {}


def _get_program():
    if "nc" not in _PROGRAM_CACHE:
        _PROGRAM_CACHE["nc"] = _build_program()
    return _PROGRAM_CACHE["nc"]


def kernel(x, weight=None, **_unused):
    x = np.ascontiguousarray(np.asarray(x), dtype=np.float32)
    assert x.shape == (B, 1, H, W), x.shape
    xs = x.reshape(B, H, W)
    band = _band_matrices()

    nc = _get_program()
    in_maps = [
        {"x": np.ascontiguousarray(xs[c * PER_CORE : (c + 1) * PER_CORE]), "band": band}
        for c in range(NCORES)
    ]
    res = run_bass_kernel_spmd(nc, in_maps, core_ids=list(range(NCORES)))
    out = np.concatenate([r["y"] for r in res.results], axis=0)
    return out.reshape(B, 1, H, W).astype(np.int32)


# revision 36
# speedup vs baseline: 1.1106x; 1.0708x over previous
"""Trainium2 Bass kernel for nn_Dilate: 7x7 all-ones conv (same padding) -> (y > 0) int32 mask.

Input  x: (16, 1, 1024, 1024) float32, weight: (1, 1, 7, 7) ones (values unused).
Output:   (16, 1, 1024, 1024) int32 in {0, 1}.

Per core (pure batch data-parallel, 2 images/core on 8 cores):
  - Row-tiles: 128 input rows (incl. 3+3 halo) -> 122 output rows.
  - Inputs load via HWDGE (sync/scalar rings, 4KB/partition descriptors
    fanned over all 16 SDMA engines) as *bitcast* float32r views - the PE
    rounds f32r internally, so no rounding op is needed anywhere.
  - Vertical 7-tap sum on TensorE: banded ones matrix [128,122] as lhsT,
    fp32r matmul at full PE rate (~13-bit mantissa, measured rel err 8e-3
    on the final 0/1 mask vs the f32 reference).
  - Horizontal 7-tap sum as one sliding-window scan on VectorE:
        state[t] = (V[t] + state) - Vpad[t-7]
    (Vpad = V with 7 leading + 3 trailing zero columns, copied PSUM->SBUF
    by ScalarE; the ISA forbids two PSUM scan operands.)  Column t holds
    the boxsum for output j = t-3, edges included via the zero pads.
  - Threshold to int8 {1,0}: ACT sigmoid(1e8*d) + round-to-nearest int
    cast (decision boundary exactly at d=0); the last two tiles use DVE
    tensor_scalar is_gt so the kernel tail never waits on ACT.
  - int8 masks (2.1MB/core) leave via GpSimd SWDGE; the host widens to
    int32.  (HWDGE packs contiguous-HBM dests onto ~2 SDMA engines, and
    int32 masks would quadruple output DMA bytes.)
"""

import numpy as np

import concourse.bacc as bacc
import concourse.mybir as mybir
from concourse.tile import TileContext
from concourse.bass_utils import run_bass_kernel_spmd

B, H, W = 16, 1024, 1024
NCORES = 8
PER_CORE = B // NCORES  # 2 images per core
R = 7
PAD = R // 2  # 3
P = 128             # SBUF partitions per tile (input rows incl. halo)
MOUT = P - (R - 1)  # 122 output rows per tile
NTILES = -(-H // MOUT)  # 9 row tiles per image

SIG_SCALE = 1.0e8    # pre-scale for the sigmoid threshold trick
N_DVE_THRESH = 1000  # disabled: ACT keeps pace now that V-copies outrank sigmoids
N_VSB = 8            # rotating once-zeroed Vpad buffers


def _band_matrices() -> np.ndarray:
    """bands[0]: t=0 (partition p = image row p, top clamp);
    bands[1]: interior (partition p = row o0-3+p);
    bands[2]: last tile (partition p = row H-128+p, bottom clamp).
    band[k, m] = 1 iff output row m sums input partition k."""
    bands = np.zeros((3, P, MOUT), dtype=np.float32)
    for m in range(MOUT):
        bands[0, max(0, m - PAD) : m + PAD + 1, m] = 1.0
        bands[1, m : m + R, m] = 1.0
    # last tile: outputs start at row H-48 = partition 80
    for m in range(48):
        bands[2, 80 + m - PAD : min(80 + m + PAD + 1, P), m] = 1.0
    return bands


def _build_program():
    nc = bacc.Bacc("TRN2")
    x_d = nc.dram_tensor("x", [PER_CORE, H, W], mybir.dt.float32, kind="ExternalInput")
    band_d = nc.dram_tensor("band", [3, P, MOUT], mybir.dt.float32r, kind="ExternalInput")
    y_d = nc.dram_tensor("y", [PER_CORE, H, W], mybir.dt.int8, kind="ExternalOutput")

    gt = mybir.AluOpType.is_gt
    sig = mybir.ActivationFunctionType.Sigmoid
    f32r = mybir.dt.float32r

    with TileContext(nc) as tc:
        with (
            tc.tile_pool(name="const", bufs=1) as cpool,
            tc.tile_pool(name="xin", bufs=8) as xpool,
            tc.tile_pool(name="dbuf", bufs=6) as dpool,
            tc.tile_pool(name="mask", bufs=6) as mpool,
            tc.tile_pool(name="psum", bufs=4, space="PSUM") as psum_pool,
        ):
            band_ts = []
            for i in range(3):
                bt = cpool.tile([P, MOUT], f32r, tag=f"band{i}")
                nc.scalar.dma_start(out=bt[:], in_=band_d[i])
                band_ts.append(bt)

            # Rotating V buffers with 7 leading and 3 trailing zero columns
            # (zeroed once; the ACT copy always writes cols 7..7+W), so one
            # scan of length W+3 covers every output column incl. edges.
            vsb = []
            for i in range(N_VSB):
                vt = cpool.tile([P, R + W + PAD], mybir.dt.float32, tag=f"vsb{i}")
                nc.gpsimd.memset(vt[:MOUT, 0:R], 0.0)
                nc.gpsimd.memset(vt[:MOUT, R + W : R + W + PAD], 0.0)
                vsb.append(vt)

            # Pre-emit every input load (highest scheduler priority ->
            # depth-8 prefetch through the xin pool; all on the otherwise
            # idle sync HWDGE ring so issues never queue behind compute).
            tiles = []
            for img in range(PER_CORE):
                for t in range(NTILES):
                    o0 = t * MOUT
                    if t == 0:
                        lo = 0
                    elif t == NTILES - 1:
                        lo = H - P
                    else:
                        lo = o0 - PAD
                    nvalid = min(MOUT, H - o0)
                    tiles.append((0 if t == 0 else (2 if t == NTILES - 1 else 1),
                                  [(img, lo, 0, P)], [(img, o0, nvalid, 0)]))
            x_tiles = []
            for band_idx, loads, stores in tiles:
                x_t = xpool.tile([P, W], f32r)
                # full 128-row HWDGE load, bitcast both sides to f32r
                # (no cast - the PE rounds internally; edge clamping is
                # baked into the per-tile band matrices so no partition
                # ever needs zeroing)
                for img, row_lo, part_lo, nrows in loads:
                    nc.sync.dma_start(
                        out=x_t[part_lo : part_lo + nrows, :],
                        in_=x_d[img, row_lo : row_lo + nrows, :].bitcast(f32r),
                    )
                x_tiles.append(x_t)

            # Software pipeline with lookahead: emit MM + V-copy for tile
            # i+LA before the scan of tile i, so ACT copies outrank the
            # sigmoids the scheduler would otherwise prefer (program order =
            # priority).  LA < N_VSB keeps the rotating-buffer RAW tracking
            # honest.
            LA = 4
            n_total = len(tiles)

            def emit_mm_copy(i):
                x_t = x_tiles[i]
                bt = band_ts[tiles[i][0]]
                v_ps = psum_pool.tile([MOUT, W], mybir.dt.float32)
                for j in range(2):
                    nc.tensor.matmul(
                        v_ps[:, j * 512 : (j + 1) * 512],
                        bt[:],
                        x_t[:, j * 512 : (j + 1) * 512],
                        start=True,
                        stop=True,
                    )
                nc.scalar.copy(vsb[i % N_VSB][:MOUT, R : R + W], v_ps[:])

            for i in range(min(LA, n_total)):
                emit_mm_copy(i)

            for tile_idx, (band_idx, loads, stores) in enumerate(tiles):
                    if tile_idx + LA < n_total:
                        emit_mm_copy(tile_idx + LA)
                    v_sb = vsb[tile_idx % N_VSB]

                    # Sliding 7-sum over [0, W+PAD): d_t[:, t'] = boxsum(j = t'-3)
                    #   state = (Vpadded[t'] + state) - Vpadded[t'-7]
                    d_t = dpool.tile([P, W + PAD], mybir.dt.float32)
                    nc.vector.tensor_tensor_scan(
                        d_t[:MOUT, :],
                        v_sb[:MOUT, R : R + W + PAD],
                        v_sb[:MOUT, 0 : W + PAD],
                        0.0,
                        mybir.AluOpType.add,
                        mybir.AluOpType.subtract,
                    )

                    # threshold: mask[j] = boxsum(j) > 0 -> int8, one op
                    m_t = mpool.tile([P, W], mybir.dt.int8)
                    if tile_idx == n_total - 1:  # final tile only: DVE ts beats ACT sigmoid on the tail chain, and an earlier DVE threshold would outrank the last scan in scheduler priority
                        nc.vector.tensor_scalar(
                            m_t[:MOUT, :], d_t[:MOUT, PAD : W + PAD], 0.0, None, gt
                        )
                    else:
                        nc.scalar.activation(
                            m_t[:MOUT, :], d_t[:MOUT, PAD : W + PAD],
                            sig, scale=SIG_SCALE,
                        )

                    # int8 SWDGE out (2.1MB/core total)
                    for img, out_row, nrows, mrow in stores:
                        nc.gpsimd.dma_start(
                            out=y_d[img, out_row : out_row + nrows, :],
                            in_=m_t[mrow : mrow + nrows, :],
                        )

    nc.compile()
    return nc


_PROGRAM_CACHE = {}


def _get_program():
    if "nc" not in _PROGRAM_CACHE:
        _PROGRAM_CACHE["nc"] = _build_program()
    return _PROGRAM_CACHE["nc"]


def kernel(x, weight=None, **_unused):
    x = np.ascontiguousarray(np.asarray(x), dtype=np.float32)
    assert x.shape == (B, 1, H, W), x.shape
    xs = x.reshape(B, H, W)
    band = _band_matrices()

    nc = _get_program()
    in_maps = [
        {"x": np.ascontiguousarray(xs[c * PER_CORE : (c + 1) * PER_CORE]), "band": band}
        for c in range(NCORES)
    ]
    res = run_bass_kernel_spmd(nc, in_maps, core_ids=list(range(NCORES)))
    out = np.concatenate([r["y"] for r in res.results], axis=0)
    return out.reshape(B, 1, H, W).astype(np.int32)

